# revision 19
# baseline (speedup 1.0000x reference)
"""AdaLoRA linear layer on 8 TRN2 NeuronCores — mixed fp8/bf16 PE path.

Computes y = x @ (W + s * (P*Lambda*mask) @ Q)^T for
x[8192,4096], W[4096,4096], P[4096,64], Q[64,4096], s=2.0.

Data-parallel over tokens (1024/core). The contraction dim is split:
the first KF=10 k-blocks (1280 of 4096) run as fp8e4 DoubleRow matmuls
(2 k-blocks per instruction, 2x PE throughput — measured 216ns per
K=256 x 512 instr, same as one bf16 K=128 instr), the remaining 22
k-blocks run in bf16. Measured end-to-end rel err 1.773e-2 on the
reference inputs (gate: 2e-2, deterministic — HW matches the host-side
quantization model to 1e-5); fp8 quantization error scales with
sqrt(KF/KB), so KF=10 keeps an 11% margin.

Scale folding so one PSUM accumulation group stays consistent:
  W is pre-scaled x32 on both the fp8 and bf16 sides (fp8 needs it to
  stay in e4m3 normal range; bf16 absorbs it exactly), Q x64, and
  Ptilde = P*(s*Lambda*mask) enters as Ptilde*32/64; the final
  psum->SBUF copy multiplies by 1/32 on the Activation engine.
"""

import os
import sys
import time
import types

for _p in ("/opt/trn_rl_repo", "/opt/pypackages"):
    if os.path.isdir(_p) and _p not in sys.path:
        sys.path.append(_p)

try:
    import antenv.axon_hooks  # noqa: F401
except Exception:
    _mod = types.ModuleType("antenv.axon_hooks")
    _mod._hook = None

    def _set_hook(h, _m=_mod):
        _m._hook = h

    def _get_hook(_m=_mod):
        return _m._hook

    _mod.set_axon_ntff_profile_hook = _set_hook
    _mod.get_axon_ntff_profile_hook = _get_hook
    try:
        from trn_agent_boot.trn_boot import _ntff_profile_via_ctypes

        _mod._hook = _ntff_profile_via_ctypes("/opt/axon/libaxon_pjrt.so")
    except Exception:
        pass
    sys.modules["antenv.axon_hooks"] = _mod

import ml_dtypes
import numpy as np

import concourse.mybir as mybir
import concourse.tile as tile
from concourse import bacc
from concourse.bass_utils import run_bass_kernel_spmd
from concourse.tile_rust import add_dep_helper

N_CORES = 8
IN_F = 4096
OUT_F = 4096
RANK = 64
BT = 8192
M_PER = BT // N_CORES
SCALING = 2.0

P_DIM = 128
KB = IN_F // P_DIM  # 32 k-blocks
KF = 12  # k-blocks in fp8 DoubleRow (must be even)
KFP = KF // 2  # DR instructions per group
KR = KB - KF  # bf16 k-blocks
# bf16 kb-range split into 3 DMA slabs (also the startup interleave
# granularity); sizes need not be equal.
KSL = [(0, KR // 3), (KR // 3, 2 * KR // 3), (2 * KR // 3, KR)]
MS = M_PER // P_DIM
N_STRIPE = 512
NS = OUT_F // N_STRIPE

XH = 512
NXC = M_PER // XH
MPC = XH // P_DIM

WSCALE = 32.0
QSCALE = 64.0

BF16 = ml_dtypes.bfloat16
F8 = ml_dtypes.float8_e4m3

_graph_cache = None


def _build_graph():
    f32 = mybir.dt.float32
    bf16 = mybir.dt.bfloat16
    f8 = mybir.dt.float8e4
    DR = mybir.MatmulPerfMode.DoubleRow

    nc = bacc.Bacc(None, target_bir_lowering=False, debug=False)

    x8d = nc.declare_dram_parameter("x8", [NXC, P_DIM, KFP, 2, XH], f8, isOutput=False)
    x16d = nc.declare_dram_parameter("x16", [NXC, P_DIM, KR, XH], bf16, isOutput=False)
    w8d = nc.declare_dram_parameter(
        "w8", [NS, P_DIM, KFP, 2, N_STRIPE], f8, isOutput=False
    )
    w16d = nc.declare_dram_parameter(
        "w16", [NS, P_DIM, KR, N_STRIPE], bf16, isOutput=False
    )
    qt8d = nc.declare_dram_parameter("qt8", [P_DIM, KFP, 2, RANK], f8, isOutput=False)
    qt16d = nc.declare_dram_parameter("qt16", [P_DIM, KR, RANK], bf16, isOutput=False)
    ptd = nc.declare_dram_parameter("pt", [P_DIM, OUT_F], bf16, isOutput=False)
    out = nc.declare_dram_parameter("out", [M_PER, OUT_F], f32, isOutput=True)

    with tile.TileContext(nc) as tc:
        with (
            tc.tile_pool(name="const", bufs=1) as constp,
            tc.tile_pool(name="xpool", bufs=1) as xpool,
            tc.tile_pool(name="wpool", bufs=2) as wpool,
            tc.tile_pool(name="ypool", bufs=3) as ypool,
            tc.tile_pool(name="psum_y", bufs=6, space="PSUM") as psum_y_pool,
            tc.tile_pool(name="psum_t", bufs=2, space="PSUM") as psum_t_pool,
        ):
            # ---- tiles ----
            qt8_sb = constp.tile([P_DIM, KFP, 2, RANK], f8)
            qt16_sb = constp.tile([P_DIM, KR, RANK], bf16)
            pt_sb = constp.tile([P_DIM, OUT_F], bf16)
            tT_all = constp.tile([P_DIM, NXC, XH], bf16)
            nc.vector.memset(tT_all[:], 0.0)

            x8s, x16s = [], []
            for h in range(NXC):
                x8_h = xpool.tile(
                    [P_DIM, KFP, 2, XH], f8, name=f"x8_h{h}", tag=f"x8_h{h}"
                )
                x16_h = xpool.tile(
                    [P_DIM, KR, XH], bf16, name=f"x16_h{h}", tag=f"x16_h{h}"
                )
                x8s.append(x8_h)
                x16s.append(x16_h)
            w8s, w16s = [], []
            for ns in range(NS):
                w8_sb = wpool.tile(
                    [P_DIM, KFP, 2, N_STRIPE], f8, tag="w8_sb", name=f"w8_sb{ns}"
                )
                w16_sb = wpool.tile(
                    [P_DIM, KR, N_STRIPE], bf16, tag="w16_sb", name=f"w16_sb{ns}"
                )
                w8s.append(w8_sb)
                w16s.append(w16_sb)

            # ---- input DMAs on gpsimd, paced pairwise with the PE's
            # kb-slab consumption during startup ----

            def x8slab(h, half=None):
                if half is None:
                    return nc.gpsimd.dma_start(out=x8s[h][:], in_=x8d[h])
                sl = slice(half * (KFP // 2), (half + 1) * (KFP // 2))
                return nc.gpsimd.dma_start(
                    out=x8s[h][:, sl, :, :], in_=x8d[h, :, sl, :, :]
                )

            def x16slab(h, s_):
                sl = slice(*KSL[s_])
                return nc.gpsimd.dma_start(
                    out=x16s[h][:, sl, :], in_=x16d[h, :, sl, :]
                )

            def w8slab(ns, half=None):
                if half is None:
                    return nc.gpsimd.dma_start(out=w8s[ns][:], in_=w8d[ns])
                sl = slice(half * (KFP // 2), (half + 1) * (KFP // 2))
                return nc.gpsimd.dma_start(
                    out=w8s[ns][:, sl, :, :], in_=w8d[ns, :, sl, :, :]
                )

            def w16slab(ns, s_):
                sl = slice(*KSL[s_])
                return nc.gpsimd.dma_start(
                    out=w16s[ns][:, sl, :], in_=w16d[ns, :, sl, :]
                )

            # chunk0/stripe0 fp8 slabs halved so the PE's first matmul only
            # waits for ~0.8MB — engine startup, not data, becomes the
            # binding constraint.
            x0 = [x8slab(0, 0), x8slab(0, 1), None, None, None]
            w0 = [w8slab(0, 0), w8slab(0, 1), None, None, None]
            dma_qt8 = nc.gpsimd.dma_start(out=qt8_sb[:], in_=qt8d[:])
            add_dep_helper(x0[1].ins, x0[0].ins, reason="pace x0b")
            dma_qt16 = None
            for s_ in range(3):
                x0[s_ + 2] = x16slab(0, s_)
                w0[s_ + 2] = w16slab(0, s_)
                add_dep_helper(x0[s_ + 2].ins, x0[s_ + 1].ins, reason="pace x0")
                if s_ == 0:
                    # qt16 (0.33MB) is first read by t-phase(0) part 1 at
                    # ~+12us; issuing it behind the first bf16 slab pair
                    # keeps it out of the startup-critical DMA window.
                    dma_qt16 = nc.gpsimd.dma_start(out=qt16_sb[:], in_=qt16d[:])
                    add_dep_helper(
                        dma_qt16.ins, x0[1].ins, reason="qt16 after x8"
                    )
            x1 = [x8slab(1), x16slab(1, 0)]
            add_dep_helper(x1[0].ins, x0[3].ins, reason="pace x1")
            w1 = [w8slab(1), w16slab(1, 0)]
            add_dep_helper(w1[0].ins, w0[4].ins, reason="pace w1")
            x1 += [x16slab(1, 1), x16slab(1, 2)]
            add_dep_helper(x1[2].ins, x0[4].ins, reason="pace x1c")
            dma_pt = nc.gpsimd.dma_start(out=pt_sb[:], in_=ptd[:])
            add_dep_helper(dma_pt.ins, x1[0].ins, reason="pace pt")
            w1 += [w16slab(1, 1), w16slab(1, 2)]
            add_dep_helper(w1[2].ins, w1[0].ins, reason="pace w1c")
            wtail = list(w1)
            for ns in range(2, NS):
                for d in (
                    w8slab(ns),
                    w16slab(ns, 0),
                    w16slab(ns, 1),
                    w16slab(ns, 2),
                ):
                    add_dep_helper(
                        d.ins, wtail[-3].ins, reason="dma window order"
                    )
                    wtail.append(d)

            # ---- compute ----
            t_psums = [None, None]

            def t_part(h, part):
                # part 0: fp8 DR k-blocks; 1/2/3: bf16 kb slabs; 3 closes.
                if part == 0:
                    t_psums[h] = psum_t_pool.tile(
                        [RANK, XH], f32, tag="psum_tT", name=f"psum_tT{h}"
                    )
                    for j in range(KFP):
                        nc.tensor.matmul(
                            t_psums[h][:],
                            lhsT=qt8_sb[:, j, :, :],
                            rhs=x8s[h][:, j, :, :],
                            start=(j == 0),
                            stop=False,
                            perf_mode=DR,
                        )
                else:
                    for kb in range(*KSL[part - 1]):
                        nc.tensor.matmul(
                            t_psums[h][:],
                            lhsT=qt16_sb[:, kb, :],
                            rhs=x16s[h][:, kb, :],
                            start=False,
                            stop=(kb == KR - 1),
                        )
                    if part == 3:
                        nc.scalar.copy(out=tT_all[0:RANK, h, :], in_=t_psums[h][:])

            def mg_8(ws, ms, ypsum):
                h, mo = divmod(ms, MPC)
                msl = slice(mo * P_DIM, (mo + 1) * P_DIM)
                for j in range(KFP):
                    nc.tensor.matmul(
                        ypsum[:],
                        lhsT=x8s[h][:, j, :, msl],
                        rhs=w8s[ws][:, j, :, :],
                        start=(j == 0),
                        stop=False,
                        perf_mode=DR,
                    )

            def mg_16(ws, ms, ypsum, k0, k1):
                h, mo = divmod(ms, MPC)
                msl = slice(mo * P_DIM, (mo + 1) * P_DIM)
                for kb in range(k0, k1):
                    nc.tensor.matmul(
                        ypsum[:],
                        lhsT=x16s[h][:, kb, msl],
                        rhs=w16s[ws][:, kb, :],
                        start=False,
                        stop=False,
                    )

            def mg_tail(ws, ms, ypsum):
                h, mo = divmod(ms, MPC)
                msl = slice(mo * P_DIM, (mo + 1) * P_DIM)
                off = ws * N_STRIPE
                nc.tensor.matmul(
                    ypsum[:],
                    lhsT=tT_all[:, h, msl],
                    rhs=pt_sb[:, off : off + N_STRIPE],
                    start=False,
                    stop=True,
                )
                y_sb = ypool.tile([P_DIM, N_STRIPE], f32, tag="y_sb", name="y_sb")
                nc.scalar.mul(y_sb[:], ypsum[:], 1.0 / WSCALE)
                nc.sync.dma_start(
                    out=out[ms * P_DIM : (ms + 1) * P_DIM, off : off + N_STRIPE],
                    in_=y_sb[:],
                )

            def new_ypsum():
                return psum_y_pool.tile(
                    [P_DIM, N_STRIPE], f32, tag="ypsum", name="ypsum"
                )

            def mg_full(ws, ms):
                yp = new_ypsum()
                mg_8(ws, ms, yp)
                mg_16(ws, ms, yp, 0, KR)
                mg_tail(ws, ms, yp)

            # PE pre-warm on the zeroed tT region while first DMAs fly.
            warm_ps = psum_t_pool.tile(
                [RANK, XH], f32, tag="psum_tT", name="warm_ps"
            )
            for _ in range(12):
                nc.tensor.matmul(
                    warm_ps[:],
                    lhsT=tT_all[:, 0, 0:RANK],
                    rhs=tT_all[:, 0, :],
                    start=True,
                    stop=True,
                )

            # Startup: interleave the four chunk-0 m-groups slab-wise so PE
            # consumption matches DMA delivery; t-phase(0) rides along.
            yps = [new_ypsum() for _ in range(MPC)]
            for ms in range(MPC):
                mg_8(0, ms, yps[ms])
            t_part(0, 0)
            for q in range(1, 4):
                for ms in range(MPC):
                    mg_16(0, ms, yps[ms], *KSL[q - 1])
                t_part(0, q)
            for ms in range(MPC):
                mg_tail(0, ms, yps[ms])

            yp04 = new_ypsum()
            mg_8(0, 4, yp04)
            t_part(1, 0)
            mg_16(0, 4, yp04, 0, KR // 2)
            t_part(1, 1)
            t_part(1, 2)
            mg_16(0, 4, yp04, KR // 2, KR)
            t_part(1, 3)
            mg_tail(0, 4, yp04)

            for ms in range(5, MS):
                mg_full(0, ms)
            for ws in range(1, NS):
                for ms in range(MS):
                    mg_full(ws, ms)

    nc.compile()
    return nc


def _get_graph():
    global _graph_cache
    if _graph_cache is None:
        _graph_cache = _build_graph()
    return _graph_cache


def _prep_inputs(inputs):
    """Host-side: fold scales, cast to fp8/bf16, pre-tile to SBUF layout."""
    x = np.asarray(inputs["x"], dtype=np.float32)
    weight = np.asarray(inputs["weight"], dtype=np.float32)
    P = np.asarray(inputs["P"], dtype=np.float32)
    Lambda = np.asarray(inputs["Lambda"], dtype=np.float32)
    Q = np.asarray(inputs["Q"], dtype=np.float32)
    rank_mask = np.asarray(inputs["rank_mask"])

    KFE = KF * P_DIM  # 1024 leading k-elements in fp8

    scale = (SCALING * Lambda * rank_mask.astype(np.float32)).astype(np.float32)
    ptil = (P * scale[None, :]).T * (WSCALE / QSCALE)  # [RANK, OUT_F]
    pt = np.zeros((P_DIM, OUT_F), dtype=BF16)
    pt[:RANK] = ptil.astype(BF16)

    q64 = (Q * QSCALE).T  # [IN_F, RANK]
    qt8 = np.ascontiguousarray(
        q64[:KFE].astype(F8).reshape(KFP, 2, P_DIM, RANK).transpose(2, 0, 1, 3)
    )
    qt16 = np.ascontiguousarray(
        q64[KFE:].astype(BF16).reshape(KR, P_DIM, RANK).transpose(1, 0, 2)
    )

    ws = (weight * WSCALE).T  # [IN_F, OUT_F]
    w8 = np.ascontiguousarray(
        ws[:KFE]
        .astype(F8)
        .reshape(KFP, 2, P_DIM, NS, N_STRIPE)
        .transpose(3, 2, 0, 1, 4)
    )
    w16 = np.ascontiguousarray(
        ws[KFE:]
        .astype(BF16)
        .reshape(KR, P_DIM, NS, N_STRIPE)
        .transpose(2, 1, 0, 3)
    )

    in_maps = []
    for c in range(N_CORES):
        xc = x[c * M_PER : (c + 1) * M_PER]  # [1024, 4096]
        xct = xc.T  # [IN_F, 1024]
        x8c = np.ascontiguousarray(
            xct[:KFE]
            .astype(F8)
            .reshape(KFP, 2, P_DIM, NXC, XH)
            .transpose(3, 2, 0, 1, 4)
        )
        x16c = np.ascontiguousarray(
            xct[KFE:]
            .astype(BF16)
            .reshape(KR, P_DIM, NXC, XH)
            .transpose(2, 1, 0, 3)
        )
        in_maps.append(
            {
                "x8": x8c,
                "x16": x16c,
                "w8": w8,
                "w16": w16,
                "qt8": qt8,
                "qt16": qt16,
                "pt": pt,
            }
        )
    return in_maps


def run_full(inputs, trace=False, trace_kwargs=None):
    """Run the SPMD kernel on 8 cores. Returns (y_full, BassKernelResults)."""
    in_maps = _prep_inputs(inputs)

    nc = _get_graph()
    last_err = None
    for attempt in range(3):
        try:
            res = run_bass_kernel_spmd(
                nc,
                in_maps,
                core_ids=list(range(N_CORES)),
                trace=trace,
                **(trace_kwargs or {}),
            )
            break
        except Exception as e:
            last_err = e
            time.sleep(10)
    else:
        raise last_err
    y = np.concatenate([res.results[c]["out"] for c in range(N_CORES)], axis=0)
    return y.astype(np.float32, copy=False), res


def _device_available():
    try:
        import jax

        return any("NC" in str(d) or "axon" in str(d).lower() for d in jax.devices())
    except Exception:
        return False


def _run_in_subprocess(inputs):
    import pickle
    import subprocess
    import tempfile

    with tempfile.TemporaryDirectory() as td:
        in_path = os.path.join(td, "in.pkl")
        out_path = os.path.join(td, "out.npy")
        with open(in_path, "wb") as f:
            pickle.dump({k: np.asarray(v) for k, v in inputs.items()}, f)
        env = dict(os.environ)
        env.pop("JAX_PLATFORMS", None)
        env["KERNEL_NO_SUBPROC"] = "1"
        code = (
            "import sys, pickle, numpy as np; "
            f"sys.path.insert(0, {os.path.dirname(os.path.abspath(__file__))!r}); "
            "import kernel; "
            f"inputs = pickle.load(open({in_path!r}, 'rb')); "
            "y, _ = kernel.run_full(inputs, trace=False); "
            f"np.save({out_path!r}, y)"
        )
        subprocess.run([sys.executable, "-c", code], env=env, check=True)
        return np.load(out_path)


def kernel(**inputs) -> np.ndarray:
    if os.environ.get("KERNEL_NO_SUBPROC") != "1":
        if not _device_available():
            return _run_in_subprocess(inputs)
        try:
            y, _ = run_full(inputs, trace=False)
            return y
        except Exception:
            return _run_in_subprocess(inputs)
    y, _ = run_full(inputs, trace=False)
    return y


# revision 20
# speedup vs baseline: 1.1922x; 1.1922x over previous
"""AdaLoRA linear layer on 8 TRN2 NeuronCores — mixed fp8/bf16 PE path.

Computes y = x @ (W + s * (P*Lambda*mask) @ Q)^T for
x[8192,4096], W[4096,4096], P[4096,64], Q[64,4096], s=2.0.

Data-parallel over tokens (1024/core). The contraction dim is split:
the first KF=10 k-blocks (1280 of 4096) run as fp8e4 DoubleRow matmuls
(2 k-blocks per instruction, 2x PE throughput — measured 216ns per
K=256 x 512 instr, same as one bf16 K=128 instr), the remaining 22
k-blocks run in bf16. Measured end-to-end rel err 1.773e-2 on the
reference inputs (gate: 2e-2, deterministic — HW matches the host-side
quantization model to 1e-5); fp8 quantization error scales with
sqrt(KF/KB), so KF=10 keeps an 11% margin.

Scale folding so one PSUM accumulation group stays consistent:
  W is pre-scaled x32 on both the fp8 and bf16 sides (fp8 needs it to
  stay in e4m3 normal range; bf16 absorbs it exactly), Q x64, and
  Ptilde = P*(s*Lambda*mask) enters as Ptilde*32/64; the final
  psum->SBUF copy multiplies by 1/32 on the Activation engine.
"""

import os
import sys
import time
import types

for _p in ("/opt/trn_rl_repo", "/opt/pypackages"):
    if os.path.isdir(_p) and _p not in sys.path:
        sys.path.append(_p)

try:
    import antenv.axon_hooks  # noqa: F401
except Exception:
    _mod = types.ModuleType("antenv.axon_hooks")
    _mod._hook = None

    def _set_hook(h, _m=_mod):
        _m._hook = h

    def _get_hook(_m=_mod):
        return _m._hook

    _mod.set_axon_ntff_profile_hook = _set_hook
    _mod.get_axon_ntff_profile_hook = _get_hook
    try:
        from trn_agent_boot.trn_boot import _ntff_profile_via_ctypes

        _mod._hook = _ntff_profile_via_ctypes("/opt/axon/libaxon_pjrt.so")
    except Exception:
        pass
    sys.modules["antenv.axon_hooks"] = _mod

import ml_dtypes
import numpy as np

import concourse.mybir as mybir
import concourse.tile as tile
from concourse import bacc
from concourse.bass_utils import run_bass_kernel_spmd
from concourse.tile_rust import add_dep_helper

N_CORES = 8
IN_F = 4096
OUT_F = 4096
RANK = 64
BT = 8192
M_PER = BT // N_CORES
SCALING = 2.0

P_DIM = 128
KB = IN_F // P_DIM  # 32 k-blocks
KF = 12  # k-blocks in fp8 DoubleRow (must be even)
KFP = KF // 2  # DR instructions per group
KR = KB - KF  # bf16 k-blocks
# bf16 kb-range split into 3 DMA slabs (also the startup interleave
# granularity); sizes need not be equal.
KSL = [(0, KR // 3), (KR // 3, 2 * KR // 3), (2 * KR // 3, KR)]
MS = M_PER // P_DIM
N_STRIPE = 512
NS = OUT_F // N_STRIPE

XH = 512
NXC = M_PER // XH
MPC = XH // P_DIM

WSCALE = 32.0
QSCALE = 64.0

BF16 = ml_dtypes.bfloat16
F8 = ml_dtypes.float8_e4m3

_graph_cache = None


def _build_graph():
    f32 = mybir.dt.float32
    bf16 = mybir.dt.bfloat16
    f8 = mybir.dt.float8e4
    DR = mybir.MatmulPerfMode.DoubleRow

    nc = bacc.Bacc(None, target_bir_lowering=False, debug=False)

    x8d = nc.declare_dram_parameter("x8", [NXC, P_DIM, KFP, 2, XH], f8, isOutput=False)
    x16d = nc.declare_dram_parameter("x16", [NXC, P_DIM, KR, XH], bf16, isOutput=False)
    w8d = nc.declare_dram_parameter(
        "w8", [NS, P_DIM, KFP, 2, N_STRIPE], f8, isOutput=False
    )
    w16d = nc.declare_dram_parameter(
        "w16", [NS, P_DIM, KR, N_STRIPE], bf16, isOutput=False
    )
    qt8d = nc.declare_dram_parameter("qt8", [P_DIM, KFP, 2, RANK], f8, isOutput=False)
    qt16d = nc.declare_dram_parameter("qt16", [P_DIM, KR, RANK], bf16, isOutput=False)
    ptd = nc.declare_dram_parameter("pt", [P_DIM, OUT_F], bf16, isOutput=False)
    out = nc.declare_dram_parameter("out", [M_PER, OUT_F], f32, isOutput=True)

    with tile.TileContext(nc) as tc:
        with (
            tc.tile_pool(name="const", bufs=1) as constp,
            tc.tile_pool(name="xpool", bufs=1) as xpool,
            tc.tile_pool(name="wpool", bufs=2) as wpool,
            tc.tile_pool(name="ypool", bufs=3) as ypool,
            tc.tile_pool(name="psum_y", bufs=6, space="PSUM") as psum_y_pool,
            tc.tile_pool(name="psum_t", bufs=2, space="PSUM") as psum_t_pool,
        ):
            # ---- tiles ----
            qt8_sb = constp.tile([P_DIM, KFP, 2, RANK], f8)
            qt16_sb = constp.tile([P_DIM, KR, RANK], bf16)
            pt_sb = constp.tile([P_DIM, OUT_F], bf16)
            tT_all = constp.tile([P_DIM, NXC, XH], bf16)
            nc.vector.memset(tT_all[:], 0.0)

            x8s, x16s = [], []
            for h in range(NXC):
                x8_h = xpool.tile(
                    [P_DIM, KFP, 2, XH], f8, name=f"x8_h{h}", tag=f"x8_h{h}"
                )
                x16_h = xpool.tile(
                    [P_DIM, KR, XH], bf16, name=f"x16_h{h}", tag=f"x16_h{h}"
                )
                x8s.append(x8_h)
                x16s.append(x16_h)
            w8s, w16s = [], []
            for ns in range(NS):
                w8_sb = wpool.tile(
                    [P_DIM, KFP, 2, N_STRIPE], f8, tag="w8_sb", name=f"w8_sb{ns}"
                )
                w16_sb = wpool.tile(
                    [P_DIM, KR, N_STRIPE], bf16, tag="w16_sb", name=f"w16_sb{ns}"
                )
                w8s.append(w8_sb)
                w16s.append(w16_sb)

            # ---- input DMAs on gpsimd, paced pairwise with the PE's
            # kb-slab consumption during startup ----

            def x8slab(h, half=None):
                if half is None:
                    return nc.gpsimd.dma_start(out=x8s[h][:], in_=x8d[h])
                sl = slice(half * (KFP // 2), (half + 1) * (KFP // 2))
                return nc.gpsimd.dma_start(
                    out=x8s[h][:, sl, :, :], in_=x8d[h, :, sl, :, :]
                )

            def x16slab(h, s_):
                sl = slice(*KSL[s_])
                return nc.gpsimd.dma_start(
                    out=x16s[h][:, sl, :], in_=x16d[h, :, sl, :]
                )

            def w8slab(ns, half=None):
                if half is None:
                    return nc.gpsimd.dma_start(out=w8s[ns][:], in_=w8d[ns])
                sl = slice(half * (KFP // 2), (half + 1) * (KFP // 2))
                return nc.gpsimd.dma_start(
                    out=w8s[ns][:, sl, :, :], in_=w8d[ns, :, sl, :, :]
                )

            def w16slab(ns, s_):
                sl = slice(*KSL[s_])
                return nc.gpsimd.dma_start(
                    out=w16s[ns][:, sl, :], in_=w16d[ns, :, sl, :]
                )

            # chunk0/stripe0 fp8 slabs halved so the PE's first matmul only
            # waits for ~0.8MB — engine startup, not data, becomes the
            # binding constraint.
            x0 = [x8slab(0, 0), x8slab(0, 1), None, None, None]
            w0 = [w8slab(0, 0), w8slab(0, 1), None, None, None]
            dma_qt8 = nc.gpsimd.dma_start(out=qt8_sb[:], in_=qt8d[:])
            dma_qt16 = nc.gpsimd.dma_start(out=qt16_sb[:], in_=qt16d[:])
            add_dep_helper(x0[1].ins, x0[0].ins, reason="pace x0b")
            for s_ in range(3):
                x0[s_ + 2] = x16slab(0, s_)
                w0[s_ + 2] = w16slab(0, s_)
                add_dep_helper(x0[s_ + 2].ins, x0[s_ + 1].ins, reason="pace x0")
            x1 = [x8slab(1), x16slab(1, 0)]
            add_dep_helper(x1[0].ins, x0[3].ins, reason="pace x1")
            w1 = [w8slab(1), w16slab(1, 0)]
            add_dep_helper(w1[0].ins, w0[4].ins, reason="pace w1")
            x1 += [x16slab(1, 1), x16slab(1, 2)]
            add_dep_helper(x1[2].ins, x0[4].ins, reason="pace x1c")
            dma_pt = nc.gpsimd.dma_start(out=pt_sb[:], in_=ptd[:])
            add_dep_helper(dma_pt.ins, x1[0].ins, reason="pace pt")
            w1 += [w16slab(1, 1), w16slab(1, 2)]
            add_dep_helper(w1[2].ins, w1[0].ins, reason="pace w1c")
            wtail = list(w1)
            for ns in range(2, NS):
                for d in (
                    w8slab(ns),
                    w16slab(ns, 0),
                    w16slab(ns, 1),
                    w16slab(ns, 2),
                ):
                    add_dep_helper(
                        d.ins, wtail[-3].ins, reason="dma window order"
                    )
                    wtail.append(d)

            # ---- compute ----
            t_psums = [None, None]

            def t_part(h, part):
                # part 0: fp8 DR k-blocks; 1/2/3: bf16 kb slabs; 3 closes.
                if part == 0:
                    t_psums[h] = psum_t_pool.tile(
                        [RANK, XH], f32, tag="psum_tT", name=f"psum_tT{h}"
                    )
                    for j in range(KFP):
                        nc.tensor.matmul(
                            t_psums[h][:],
                            lhsT=qt8_sb[:, j, :, :],
                            rhs=x8s[h][:, j, :, :],
                            start=(j == 0),
                            stop=False,
                            perf_mode=DR,
                        )
                else:
                    for kb in range(*KSL[part - 1]):
                        nc.tensor.matmul(
                            t_psums[h][:],
                            lhsT=qt16_sb[:, kb, :],
                            rhs=x16s[h][:, kb, :],
                            start=False,
                            stop=(kb == KR - 1),
                        )
                    if part == 3:
                        nc.scalar.copy(out=tT_all[0:RANK, h, :], in_=t_psums[h][:])

            def mg_8(ws, ms, ypsum):
                h, mo = divmod(ms, MPC)
                msl = slice(mo * P_DIM, (mo + 1) * P_DIM)
                for j in range(KFP):
                    nc.tensor.matmul(
                        ypsum[:],
                        lhsT=x8s[h][:, j, :, msl],
                        rhs=w8s[ws][:, j, :, :],
                        start=(j == 0),
                        stop=False,
                        perf_mode=DR,
                    )

            def mg_16(ws, ms, ypsum, k0, k1):
                h, mo = divmod(ms, MPC)
                msl = slice(mo * P_DIM, (mo + 1) * P_DIM)
                for kb in range(k0, k1):
                    nc.tensor.matmul(
                        ypsum[:],
                        lhsT=x16s[h][:, kb, msl],
                        rhs=w16s[ws][:, kb, :],
                        start=False,
                        stop=False,
                    )

            def mg_tail(ws, ms, ypsum):
                h, mo = divmod(ms, MPC)
                msl = slice(mo * P_DIM, (mo + 1) * P_DIM)
                off = ws * N_STRIPE
                nc.tensor.matmul(
                    ypsum[:],
                    lhsT=tT_all[:, h, msl],
                    rhs=pt_sb[:, off : off + N_STRIPE],
                    start=False,
                    stop=True,
                )
                y_sb = ypool.tile([P_DIM, N_STRIPE], f32, tag="y_sb", name="y_sb")
                nc.scalar.mul(y_sb[:], ypsum[:], 1.0 / WSCALE)
                nc.sync.dma_start(
                    out=out[ms * P_DIM : (ms + 1) * P_DIM, off : off + N_STRIPE],
                    in_=y_sb[:],
                )

            def new_ypsum():
                return psum_y_pool.tile(
                    [P_DIM, N_STRIPE], f32, tag="ypsum", name="ypsum"
                )

            def mg_full(ws, ms):
                yp = new_ypsum()
                mg_8(ws, ms, yp)
                mg_16(ws, ms, yp, 0, KR)
                mg_tail(ws, ms, yp)

            # PE pre-warm on the zeroed tT region while first DMAs fly.
            warm_ps = psum_t_pool.tile(
                [RANK, XH], f32, tag="psum_tT", name="warm_ps"
            )
            for _ in range(12):
                nc.tensor.matmul(
                    warm_ps[:],
                    lhsT=tT_all[:, 0, 0:RANK],
                    rhs=tT_all[:, 0, :],
                    start=True,
                    stop=True,
                )

            # Startup: interleave the four chunk-0 m-groups slab-wise so PE
            # consumption matches DMA delivery; t-phase(0) rides along.
            yps = [new_ypsum() for _ in range(MPC)]
            for ms in range(MPC):
                mg_8(0, ms, yps[ms])
            t_part(0, 0)
            for q in range(1, 4):
                for ms in range(MPC):
                    mg_16(0, ms, yps[ms], *KSL[q - 1])
                t_part(0, q)
            for ms in range(MPC):
                mg_tail(0, ms, yps[ms])

            yp04 = new_ypsum()
            mg_8(0, 4, yp04)
            t_part(1, 0)
            mg_16(0, 4, yp04, 0, KR // 2)
            t_part(1, 1)
            t_part(1, 2)
            mg_16(0, 4, yp04, KR // 2, KR)
            t_part(1, 3)
            mg_tail(0, 4, yp04)

            for ms in range(5, MS):
                mg_full(0, ms)
            for ws in range(1, NS):
                for ms in range(MS):
                    mg_full(ws, ms)

    nc.compile()
    return nc


def _get_graph():
    global _graph_cache
    if _graph_cache is None:
        _graph_cache = _build_graph()
    return _graph_cache


def _prep_inputs(inputs):
    """Host-side: fold scales, cast to fp8/bf16, pre-tile to SBUF layout."""
    x = np.asarray(inputs["x"], dtype=np.float32)
    weight = np.asarray(inputs["weight"], dtype=np.float32)
    P = np.asarray(inputs["P"], dtype=np.float32)
    Lambda = np.asarray(inputs["Lambda"], dtype=np.float32)
    Q = np.asarray(inputs["Q"], dtype=np.float32)
    rank_mask = np.asarray(inputs["rank_mask"])

    KFE = KF * P_DIM  # 1024 leading k-elements in fp8

    scale = (SCALING * Lambda * rank_mask.astype(np.float32)).astype(np.float32)
    ptil = (P * scale[None, :]).T * (WSCALE / QSCALE)  # [RANK, OUT_F]
    pt = np.zeros((P_DIM, OUT_F), dtype=BF16)
    pt[:RANK] = ptil.astype(BF16)

    q64 = (Q * QSCALE).T  # [IN_F, RANK]
    qt8 = np.ascontiguousarray(
        q64[:KFE].astype(F8).reshape(KFP, 2, P_DIM, RANK).transpose(2, 0, 1, 3)
    )
    qt16 = np.ascontiguousarray(
        q64[KFE:].astype(BF16).reshape(KR, P_DIM, RANK).transpose(1, 0, 2)
    )

    ws = (weight * WSCALE).T  # [IN_F, OUT_F]
    w8 = np.ascontiguousarray(
        ws[:KFE]
        .astype(F8)
        .reshape(KFP, 2, P_DIM, NS, N_STRIPE)
        .transpose(3, 2, 0, 1, 4)
    )
    w16 = np.ascontiguousarray(
        ws[KFE:]
        .astype(BF16)
        .reshape(KR, P_DIM, NS, N_STRIPE)
        .transpose(2, 1, 0, 3)
    )

    in_maps = []
    for c in range(N_CORES):
        xc = x[c * M_PER : (c + 1) * M_PER]  # [1024, 4096]
        xct = xc.T  # [IN_F, 1024]
        x8c = np.ascontiguousarray(
            xct[:KFE]
            .astype(F8)
            .reshape(KFP, 2, P_DIM, NXC, XH)
            .transpose(3, 2, 0, 1, 4)
        )
        x16c = np.ascontiguousarray(
            xct[KFE:]
            .astype(BF16)
            .reshape(KR, P_DIM, NXC, XH)
            .transpose(2, 1, 0, 3)
        )
        in_maps.append(
            {
                "x8": x8c,
                "x16": x16c,
                "w8": w8,
                "w16": w16,
                "qt8": qt8,
                "qt16": qt16,
                "pt": pt,
            }
        )
    return in_maps


def run_full(inputs, trace=False, trace_kwargs=None):
    """Run the SPMD kernel on 8 cores. Returns (y_full, BassKernelResults)."""
    in_maps = _prep_inputs(inputs)

    nc = _get_graph()
    last_err = None
    for attempt in range(3):
        try:
            res = run_bass_kernel_spmd(
                nc,
                in_maps,
                core_ids=list(range(N_CORES)),
                trace=trace,
                **(trace_kwargs or {}),
            )
            break
        except Exception as e:
            last_err = e
            time.sleep(10)
    else:
        raise last_err
    y = np.concatenate([res.results[c]["out"] for c in range(N_CORES)], axis=0)
    return y.astype(np.float32, copy=False), res


def _device_available():
    try:
        import jax

        return any("NC" in str(d) or "axon" in str(d).lower() for d in jax.devices())
    except Exception:
        return False


def _run_in_subprocess(inputs):
    import pickle
    import subprocess
    import tempfile

    with tempfile.TemporaryDirectory() as td:
        in_path = os.path.join(td, "in.pkl")
        out_path = os.path.join(td, "out.npy")
        with open(in_path, "wb") as f:
            pickle.dump({k: np.asarray(v) for k, v in inputs.items()}, f)
        env = dict(os.environ)
        env.pop("JAX_PLATFORMS", None)
        env["KERNEL_NO_SUBPROC"] = "1"
        code = (
            "import sys, pickle, numpy as np; "
            f"sys.path.insert(0, {os.path.dirname(os.path.abspath(__file__))!r}); "
            "import kernel; "
            f"inputs = pickle.load(open({in_path!r}, 'rb')); "
            "y, _ = kernel.run_full(inputs, trace=False); "
            f"np.save({out_path!r}, y)"
        )
        subprocess.run([sys.executable, "-c", code], env=env, check=True)
        return np.load(out_path)


def kernel(**inputs) -> np.ndarray:
    if os.environ.get("KERNEL_NO_SUBPROC") != "1":
        if not _device_available():
            return _run_in_subprocess(inputs)
        try:
            y, _ = run_full(inputs, trace=False)
            return y
        except Exception:
            return _run_in_subprocess(inputs)
    y, _ = run_full(inputs, trace=False)
    return y
